# revision 14
# baseline (speedup 1.0000x reference)
"""Trainium2 Bass kernel for nn_Attention_45406394253499.

Fused: LayerNorm -> QKV proj -> blended attention (0.25*qk*scale + 0.75*h)
-> softmax -> PV -> out proj.  Outputs (out, blended).

Sharding: data-parallel over batch. B=8 batches -> 8 NeuronCores, one batch
per core, no collectives.  Each core runs the identical program on its slice.

Core-local dataflow (all fp32, matmuls in float32r):
  x [1024,768] --LN--> xn --PE transpose--> xnT [768,1024] (feature-major)
  qT,kT = (w_qkv.T @ xnT)      [1536,1024]  (head h at rows h*64..h*64+63)
  V     = (xnT.T @ w_qkv_v)    [1024,768]   token-major, + ones col (aug)
  per head:
    raw  = qT.T @ kT           [128i,512j] tiles  (q pre-scaled by 0.25/sqrt(d))
    blended = raw + 0.75*h     (DVE, evacuates PSUM) -> DRAM + kept in SBUF
    blendedT via PE transpose -> PSUM, exp on ACT -> attnT [j,i] SBUF
    [OT;den] = [V|1].T @ attnT (PV, M=65)  ;  OT *= 1/den (broadcast)
  out = OT.T @ w_out + b_out   [1024,768]
"""

import numpy as np
from contextlib import ExitStack

import concourse.bass as bass
import concourse.tile as tile
from concourse import bacc, mybir
from concourse.bass_utils import run_bass_kernel_spmd
from concourse.masks import make_identity

B, N, DIM = 8, 1024, 768
HEADS, DHEAD = 12, 64
INNER = HEADS * DHEAD            # 768
ALPHA = 0.25
SCALE = DHEAD ** -0.5
EPS = 1e-5

F32 = mybir.dt.float32
F32R = mybir.dt.float32r
AF = mybir.ActivationFunctionType
ALU = mybir.AluOpType

NT = N // 128                    # 8 token tiles
KT = DIM // 128                  # 6 feature tiles
QK_M = 2 * INNER // 128          # 12 M-tiles for q|k rows

import os as _os
KHEADS = int(_os.environ.get("KHEADS", str(HEADS)))

# Set by test.py to collect a profile; results of last run stashed here.
TRACE = False
LAST_RESULTS = None

_cached_nc = None


def _r(ap):
    """Matmul-operand tiles are allocated natively as float32r."""
    return ap


def _build_nc() -> bass.Bass:
    nc = bacc.Bacc("TRN2", target_bir_lowering=False, debug=False)

    x_d = nc.declare_dram_parameter("x", [N, DIM], F32, isOutput=False)
    h_d = nc.declare_dram_parameter("h", [HEADS, N, N], F32, isOutput=False)
    gamma_d = nc.declare_dram_parameter("gamma", [DIM], F32, isOutput=False)
    beta_d = nc.declare_dram_parameter("beta", [DIM], F32, isOutput=False)
    wqkv_d = nc.declare_dram_parameter("w_qkv", [DIM, 3 * INNER], F32, isOutput=False)
    wout_d = nc.declare_dram_parameter("w_out", [INNER, DIM], F32, isOutput=False)
    bout_d = nc.declare_dram_parameter("b_out", [DIM], F32, isOutput=False)
    out_d = nc.declare_dram_parameter("out", [N, DIM], F32, isOutput=True)
    blend_d = nc.declare_dram_parameter("blended", [HEADS, N, N], F32, isOutput=True)
    # DRAM scratch used to broadcast the per-token 1/denom row across
    # partitions (SBUF sources cannot replicate; DRAM sources can).
    # Declared as an output: internal DRAM tensors fail NEFF load under the
    # axon PJRT path; output DRAM is allocated by the runtime and works.
    rdsc_d = nc.declare_dram_parameter("rd_scratch", [HEADS, 2, 512], F32,
                                       isOutput=True)

    with tile.TileContext(nc) as tc:
        _emit(tc, nc, x_d, h_d, gamma_d, beta_d, wqkv_d, wout_d, bout_d,
              out_d, blend_d, rdsc_d)
    nc.compile()
    return nc


def _emit(tc, nc, x_d, h_d, gamma_d, beta_d, wqkv_d, wout_d, bout_d,
          out_d, blend_d, rdsc_d):
    with ExitStack() as root:
        const = root.enter_context(tc.tile_pool(name="const", bufs=1))
        ident = const.tile([128, 128], F32)
        make_identity(nc, ident)
        eps_t = const.tile([128, 1], F32)
        nc.vector.memset(eps_t, EPS)
        # gamma/beta in feature-major layout: [p, kt] with feature = kt*128+p
        gbT = const.tile([128, KT], F32)
        bbT = const.tile([128, KT], F32)
        nc.sync.dma_start(out=gbT, in_=gamma_d[:].rearrange("(t p) -> p t", p=128))
        nc.sync.dma_start(out=bbT, in_=beta_d[:].rearrange("(t p) -> p t", p=128))
        # b_out broadcast to all partitions
        bob = const.tile([128, DIM], F32)
        _bo = bout_d[:]
        nc.gpsimd.dma_start(out=bob, in_=bass.AP(
            tensor=_bo.tensor, offset=_bo.offset, ap=[[0, 128]] + list(_bo.ap)))

        # Pools that live across phases
        qk_pool = root.enter_context(tc.tile_pool(name="qkT", bufs=QK_M))
        v_pool = root.enter_context(tc.tile_pool(name="V", bufs=1))
        ot_pool = root.enter_context(tc.tile_pool(name="OT", bufs=1))

        qkT = [qk_pool.tile([128, N], F32R, tag="qkT", name=f"qkT{m}")
               for m in range(QK_M)]
        # V augmented with a ones column: [j_part, j_tile, head, 65]
        V_sb = v_pool.tile([128, NT, HEADS, DHEAD + 1], F32R)
        ones_t = v_pool.tile([128, NT * HEADS], F32)
        nc.vector.memset(ones_t, 1.0)
        nc.vector.tensor_copy(
            V_sb[:, :, :, DHEAD], ones_t.rearrange("p (a b) -> p a b", a=NT))
        OT = ot_pool.tile([128, KT, N], F32R)

        # ---------------- Phase A: LN + QKV ----------------
        with ExitStack() as pa:
            wq_p = pa.enter_context(tc.tile_pool(name="wqkv", bufs=1))
            xnt_p = pa.enter_context(tc.tile_pool(name="xnt", bufs=1))
            x_p = pa.enter_context(tc.tile_pool(name="xin", bufs=3))
            ln_p = pa.enter_context(tc.tile_pool(name="ln", bufs=8))
            tr_ps = pa.enter_context(tc.tile_pool(name="lntr", bufs=2, space="PSUM"))
            qk_ps = pa.enter_context(tc.tile_pool(name="qkps", bufs=3, space="PSUM"))
            v_ps = pa.enter_context(tc.tile_pool(name="vps", bufs=2, space="PSUM"))

            wqkv = wq_p.tile([128, KT, 3 * INNER], F32R)
            nc.gpsimd.dma_start(
                out=wqkv, in_=wqkv_d[:].rearrange("(t p) n -> p t n", p=128))
            # Fold 0.25*SCALE into the q columns so raw comes out pre-scaled.
            nc.vector.tensor_scalar_mul(
                wqkv[:, :, 0:INNER], wqkv[:, :, 0:INNER], ALPHA * SCALE)

            xnT = xnt_p.tile([128, KT, N], F32R)

            for ti in range(NT):
                x_t = x_p.tile([128, DIM], F32, tag="x")
                nc.sync.dma_start(out=x_t, in_=x_d[ti * 128:(ti + 1) * 128, :])
                stats = ln_p.tile([128, 3, 6], F32, tag="st")
                for sg in range(3):
                    nc.vector.bn_stats(
                        out=stats[:, sg, :], in_=x_t[:, sg * 256:(sg + 1) * 256])
                mv = ln_p.tile([128, 2], F32, tag="mv")
                nc.vector.bn_aggr(out=mv, in_=stats)
                s_t = ln_p.tile([128, 1], F32, tag="s")
                nc.scalar.activation(out=s_t, in_=mv[:, 1:2], func=AF.Sqrt,
                                     bias=eps_t, scale=1.0)
                nc.vector.reciprocal(out=s_t, in_=s_t)
                tneg = ln_p.tile([128, 1], F32, tag="t")
                nc.vector.scalar_tensor_tensor(
                    out=tneg, in0=mv[:, 0:1], scalar=-1.0, in1=s_t,
                    op0=ALU.mult, op1=ALU.mult)
                xc = x_p.tile([128, DIM], F32, tag="xc")
                nc.scalar.activation(out=xc, in_=x_t, func=AF.Identity,
                                     bias=tneg, scale=s_t)
                for ft in range(KT):
                    tp = tr_ps.tile([128, 128], F32, tag="tp")
                    nc.tensor.transpose(tp, xc[:, ft * 128:(ft + 1) * 128], ident)
                    nc.scalar.activation(
                        out=xnT[:, ft, ti * 128:(ti + 1) * 128], in_=tp,
                        func=AF.Identity, bias=bbT[:, ft:ft + 1],
                        scale=gbT[:, ft:ft + 1])

            # qT / kT : out[m_tile 128, tok 512], contract over features
            for m in range(QK_M):
                for ih in range(2):
                    ps = qk_ps.tile([128, 512], F32, tag="qk")
                    for kt in range(KT):
                        nc.tensor.matmul(
                            ps,
                            _r(wqkv[:, kt, m * 128:(m + 1) * 128]),
                            _r(xnT[:, kt, ih * 512:(ih + 1) * 512]),
                            start=(kt == 0), stop=(kt == KT - 1))
                    eng = nc.vector if (m + ih) % 2 == 0 else nc.scalar
                    if eng is nc.vector:
                        nc.vector.tensor_copy(
                            qkT[m][:, ih * 512:(ih + 1) * 512], ps)
                    else:
                        nc.scalar.copy(qkT[m][:, ih * 512:(ih + 1) * 512], ps)

            # V token-major: out[tok 128, vcols], contract over features
            for ti in range(NT):
                for ci, (off, nn) in enumerate(((0, 512), (512, 256))):
                    ps = v_ps.tile([128, 512], F32, tag="v")
                    for kt in range(KT):
                        nc.tensor.matmul(
                            ps[:, 0:nn],
                            _r(xnT[:, kt, ti * 128:(ti + 1) * 128]),
                            _r(wqkv[:, kt, 2 * INNER + off:2 * INNER + off + nn]),
                            start=(kt == 0), stop=(kt == KT - 1))
                    h0 = off // DHEAD
                    nh = nn // DHEAD
                    nc.vector.tensor_copy(
                        V_sb[:, ti, h0:h0 + nh, 0:DHEAD],
                        ps[:, 0:nn].rearrange("p (a b) -> p a b", a=nh))

        # ---------------- Phase B: attention ----------------
        with ExitStack() as pb:
            h_p = pb.enter_context(tc.tile_pool(name="hin", bufs=3))
            bl_p = pb.enter_context(tc.tile_pool(name="bl", bufs=9))
            at_p = pb.enter_context(tc.tile_pool(name="at", bufs=9))
            sm_p = pb.enter_context(tc.tile_pool(name="sm", bufs=4))
            raw_ps = pb.enter_context(tc.tile_pool(name="raw", bufs=2, space="PSUM"))
            trb_ps = pb.enter_context(tc.tile_pool(name="trb", bufs=2, space="PSUM"))
            pv_ps = pb.enter_context(tc.tile_pool(name="pv", bufs=2, space="PSUM"))

            for hd in range(KHEADS):
                g, sub = divmod(hd, 2)
                po = 64 * sub
                qm, km = g, 6 + g
                bls = []
                for ti in range(NT):
                    h_t = h_p.tile([128, N], F32, tag="h")
                    nc.sync.dma_start(
                        out=h_t, in_=h_d[hd, ti * 128:(ti + 1) * 128, :])
                    bl = bl_p.tile([128, N], F32, tag="bl")
                    for jh in range(2):
                        rp = raw_ps.tile([128, 512], F32, tag="raw")
                        nc.tensor.matmul(
                            rp,
                            _r(qkT[qm][po:po + 64, ti * 128:(ti + 1) * 128]),
                            _r(qkT[km][po:po + 64, jh * 512:(jh + 1) * 512]),
                            start=True, stop=True)
                        nc.vector.scalar_tensor_tensor(
                            out=bl[:, jh * 512:(jh + 1) * 512],
                            in0=h_t[:, jh * 512:(jh + 1) * 512], scalar=0.75,
                            in1=rp, op0=ALU.mult, op1=ALU.add)
                    nc.sync.dma_start(
                        out=blend_d[hd, ti * 128:(ti + 1) * 128, :], in_=bl)
                    bls.append(bl)

                ats = []
                for jt in range(NT):
                    tp = trb_ps.tile([128, N], F32, tag="trb")
                    for ti in range(NT):
                        nc.tensor.transpose(
                            tp[:, ti * 128:(ti + 1) * 128],
                            bls[ti][:, jt * 128:(jt + 1) * 128], ident)
                    at = at_p.tile([128, N], F32R, tag="at")
                    nc.scalar.activation(out=at, in_=tp, func=AF.Exp)
                    ats.append(at)

                for ih in range(2):
                    pp = pv_ps.tile([DHEAD + 1, 512], F32, tag="pv")
                    for jt in range(NT):
                        nc.tensor.matmul(
                            pp,
                            _r(V_sb[:, jt, hd, :]),
                            _r(ats[jt][:, ih * 512:(ih + 1) * 512]),
                            start=(jt == 0), stop=(jt == NT - 1))
                    rd = sm_p.tile([1, 512], F32, tag="rd")
                    nc.vector.reciprocal(out=rd, in_=pp[DHEAD:DHEAD + 1, :])
                    nc.sync.dma_start(
                        out=rdsc_d[hd, ih, :].rearrange("(a b) -> a b", a=1),
                        in_=rd[0:1, :])
                    rdb = sm_p.tile([64, 512], F32, tag="rdb")
                    _sc = rdsc_d[hd, ih, :]
                    nc.gpsimd.dma_start(out=rdb, in_=bass.AP(
                        tensor=_sc.tensor, offset=_sc.offset,
                        ap=[[0, 64]] + list(_sc.ap)))
                    nc.vector.tensor_mul(
                        OT[po:po + 64, g, ih * 512:(ih + 1) * 512],
                        pp[0:DHEAD, :], rdb)

        # ---------------- Phase C: output projection ----------------
        with ExitStack() as pc:
            wo_p = pc.enter_context(tc.tile_pool(name="wout", bufs=1))
            o_p = pc.enter_context(tc.tile_pool(name="osb", bufs=3))
            o_ps = pc.enter_context(tc.tile_pool(name="ops", bufs=2, space="PSUM"))

            wout = wo_p.tile([128, KT, DIM], F32R)
            nc.gpsimd.dma_start(
                out=wout, in_=wout_d[:].rearrange("(t p) n -> p t n", p=128))

            for ti in range(NT):
                op = o_ps.tile([128, DIM], F32, tag="o")
                for off, nn in ((0, 512), (512, 256)):
                    for ft in range(KT):
                        nc.tensor.matmul(
                            op[:, off:off + nn],
                            _r(OT[:, ft, ti * 128:(ti + 1) * 128]),
                            _r(wout[:, ft, off:off + nn]),
                            start=(ft == 0), stop=(ft == KT - 1))
                ot = o_p.tile([128, DIM], F32, tag="ot")
                nc.vector.scalar_tensor_tensor(
                    out=ot, in0=op, scalar=1.0, in1=bob,
                    op0=ALU.mult, op1=ALU.add)
                nc.sync.dma_start(out=out_d[ti * 128:(ti + 1) * 128, :], in_=ot)


def kernel(x, h, gamma, beta, w_qkv, w_out, b_out):
    global _cached_nc, LAST_RESULTS
    x = np.asarray(x, dtype=np.float32)
    h = np.asarray(h, dtype=np.float32)
    gamma = np.asarray(gamma, dtype=np.float32)
    beta = np.asarray(beta, dtype=np.float32)
    w_qkv = np.asarray(w_qkv, dtype=np.float32)
    w_out = np.asarray(w_out, dtype=np.float32)
    b_out = np.asarray(b_out, dtype=np.float32)

    if _cached_nc is None:
        _cached_nc = _build_nc()
    nc = _cached_nc

    in_maps = []
    for b in range(B):
        in_maps.append({
            "x": np.ascontiguousarray(x[b]),
            "h": np.ascontiguousarray(h[b]),
            "gamma": gamma, "beta": beta,
            "w_qkv": w_qkv, "w_out": w_out, "b_out": b_out,
        })
    res = run_bass_kernel_spmd(nc, in_maps, list(range(B)), trace=TRACE)
    LAST_RESULTS = res
    out = np.stack([res.results[b]["out"] for b in range(B)])
    blended = np.stack([res.results[b]["blended"] for b in range(B)])
    return out, blended


# revision 25
# speedup vs baseline: 143.2503x; 143.2503x over previous
"""Trainium2 Bass kernel for nn_Attention_45406394253499.

Fused: LayerNorm -> QKV proj -> blended attention (0.25*qk*scale + 0.75*h)
-> softmax -> PV -> out proj.  Outputs (out, blended).

Sharding: data-parallel over batch. B=8 batches -> 8 NeuronCores, one batch
per core, no collectives.  Each core runs the identical program on its slice.

Core-local dataflow (fp32; matmul operands in float32r ~ tf32):
  x [1024,768] --LN--> xn --PE transpose--> xnT [768,1024] (feature-major)
  V = xnT.T @ w_qkv_v   [1024,768] token-major + ones col (denominator trick)
  per head-pair g (heads 2g, 2g+1):    <- interleaved so DMA/PE/DVE/ACT overlap
    qT_g,kT_g = w_qkv_block.T @ xnT    [128,1024] each (q pre-scaled 0.25/sqrt d)
    per head:
      raw      = qT.T @ kT             [128i,512j] PSUM tiles
      blended  = 0.75*h + raw          (DVE stt, evacuates PSUM) -> DRAM + SBUF
      per j-tile: PE-transpose 8x128x128 -> PSUM, ACT exp -> attnT [j,i] f32r,
                  2 PV matmuls accumulate [V|1].T @ attnT -> [65, 2, 512] PSUM
      OT_head = PV[0:64] * (1/PV[64])  (reciprocal broadcast via DRAM bounce)
  out = OT.T @ w_out + b_out           [1024,768]
"""

import numpy as np
from contextlib import ExitStack

import concourse.bass as bass
import concourse.tile as tile
from concourse import bacc, mybir
from concourse.bass_utils import run_bass_kernel_spmd
from concourse.masks import make_identity

B, N, DIM = 8, 1024, 768
HEADS, DHEAD = 12, 64
INNER = HEADS * DHEAD            # 768
ALPHA = 0.25
SCALE = DHEAD ** -0.5
EPS = 1e-5

F32 = mybir.dt.float32
F32R = mybir.dt.float32r
AF = mybir.ActivationFunctionType
ALU = mybir.AluOpType

NT = N // 128                    # 8 token tiles
KT = DIM // 128                  # 6 feature tiles

import os as _os
KHEADS = int(_os.environ.get("KHEADS", str(HEADS)))

TRACE = False
LAST_RESULTS = None
_cached_nc = None


def _build_nc() -> bass.Bass:
    nc = bacc.Bacc("TRN2", target_bir_lowering=False, debug=False)

    x_d = nc.declare_dram_parameter("x", [N, DIM], F32, isOutput=False)
    h_d = nc.declare_dram_parameter("h", [HEADS, N, N], F32, isOutput=False)
    gamma_d = nc.declare_dram_parameter("gamma", [DIM], F32, isOutput=False)
    beta_d = nc.declare_dram_parameter("beta", [DIM], F32, isOutput=False)
    wqkv_d = nc.declare_dram_parameter("w_qkv", [DIM, 3 * INNER], F32, isOutput=False)
    wout_d = nc.declare_dram_parameter("w_out", [INNER, DIM], F32, isOutput=False)
    bout_d = nc.declare_dram_parameter("b_out", [DIM], F32, isOutput=False)
    out_d = nc.declare_dram_parameter("out", [N, DIM], F32, isOutput=True)
    blend_d = nc.declare_dram_parameter("blended", [HEADS, N, N], F32, isOutput=True)
    # DRAM bounce buffer for broadcasting per-token 1/denom across partitions
    # (SBUF DMA sources cannot replicate; DRAM sources can). Declared as an
    # output: internal DRAM tensors fail NEFF load under the axon PJRT path.
    rdsc_d = nc.declare_dram_parameter("rd_scratch", [HEADS, N], F32,
                                       isOutput=True)

    with tile.TileContext(nc) as tc:
        _emit(tc, nc, x_d, h_d, gamma_d, beta_d, wqkv_d, wout_d, bout_d,
              out_d, blend_d, rdsc_d)
    nc.compile()
    return nc


def _emit(tc, nc, x_d, h_d, gamma_d, beta_d, wqkv_d, wout_d, bout_d,
          out_d, blend_d, rdsc_d):
    with ExitStack() as root:
        const = root.enter_context(tc.tile_pool(name="const", bufs=1))
        ident = const.tile([128, 128], F32)
        make_identity(nc, ident)
        eps_t = const.tile([128, 1], F32)
        nc.vector.memset(eps_t, EPS)
        gbT = const.tile([128, KT], F32)
        bbT = const.tile([128, KT], F32)
        bob = const.tile([128, DIM], F32)
        _bo = bout_d[:]
        nc.gpsimd.dma_start(out=bob, in_=bass.AP(
            tensor=_bo.tensor, offset=_bo.offset,
            ap=[[0, 128]] + list(_bo.ap)))
        nc.sync.dma_start(out=gbT, in_=gamma_d[:].rearrange("(t p) -> p t", p=128))
        nc.sync.dma_start(out=bbT, in_=beta_d[:].rearrange("(t p) -> p t", p=128))

        # long-lived tensors
        big = root.enter_context(tc.tile_pool(name="big", bufs=1))
        xnT = big.tile([128, KT, N], F32R)
        V_sb = big.tile([128, NT, HEADS, DHEAD + 1], F32R)
        OT = big.tile([128, KT, N], F32R)
        ones_t = const.tile([128, NT * HEADS], F32)
        nc.vector.memset(ones_t, 1.0)
        nc.vector.tensor_copy(
            V_sb[:, :, :, DHEAD], ones_t.rearrange("p (a b) -> p a b", a=NT))

        # attention-phase working pools (root scope so DMAs hoist early)
        qk_p = root.enter_context(tc.tile_pool(name="qk", bufs=4))
        bl_p = root.enter_context(tc.tile_pool(name="bl", bufs=16))
        at_p = root.enter_context(tc.tile_pool(name="at", bufs=4))
        sm_p = root.enter_context(tc.tile_pool(name="sm", bufs=2))
        wv_p = root.enter_context(tc.tile_pool(name="wv", bufs=2))

        # ---------------- LayerNorm -> xnT ----------------
        with ExitStack() as pl:
            x_p = pl.enter_context(tc.tile_pool(name="xin", bufs=2))
            ln_p = pl.enter_context(tc.tile_pool(name="ln", bufs=8))
            tr_ps = pl.enter_context(tc.tile_pool(name="lntr", bufs=2, space="PSUM"))
            for ti in range(NT):
                x_t = x_p.tile([128, DIM], F32, tag="x")
                nc.sync.dma_start(out=x_t, in_=x_d[ti * 128:(ti + 1) * 128, :])
                stats = ln_p.tile([128, 3, 6], F32, tag="st")
                for sg in range(3):
                    nc.vector.bn_stats(
                        out=stats[:, sg, :], in_=x_t[:, sg * 256:(sg + 1) * 256])
                mv = ln_p.tile([128, 2], F32, tag="mv")
                nc.vector.bn_aggr(out=mv, in_=stats)
                s_t = ln_p.tile([128, 1], F32, tag="s")
                nc.scalar.activation(out=s_t, in_=mv[:, 1:2], func=AF.Sqrt,
                                     bias=eps_t, scale=1.0)
                nc.vector.reciprocal(out=s_t, in_=s_t)
                tneg = ln_p.tile([128, 1], F32, tag="t")
                nc.vector.scalar_tensor_tensor(
                    out=tneg, in0=mv[:, 0:1], scalar=-1.0, in1=s_t,
                    op0=ALU.mult, op1=ALU.mult)
                xc = x_p.tile([128, DIM], F32, tag="xc")
                nc.scalar.activation(out=xc, in_=x_t, func=AF.Identity,
                                     bias=tneg, scale=s_t)
                for ft in range(KT):
                    tp = tr_ps.tile([128, 128], F32, tag="tp")
                    nc.tensor.transpose(tp, xc[:, ft * 128:(ft + 1) * 128], ident)
                    nc.vector.tensor_scalar(
                        out=xnT[:, ft, ti * 128:(ti + 1) * 128], in0=tp,
                        scalar1=gbT[:, ft:ft + 1], scalar2=bbT[:, ft:ft + 1],
                        op0=ALU.mult, op1=ALU.add)

        # ------- interleaved V-chunks + QKV + attention, per head pair -----
        with ExitStack() as pb:
            wqk_p = pb.enter_context(tc.tile_pool(name="wqk", bufs=2))
            qk_ps = pb.enter_context(tc.tile_pool(name="qkps", bufs=1, space="PSUM"))
            raw_ps = pb.enter_context(tc.tile_pool(name="raw", bufs=2, space="PSUM"))
            trb_ps = pb.enter_context(tc.tile_pool(name="trb", bufs=2, space="PSUM"))
            pv_ps = pb.enter_context(tc.tile_pool(name="pv", bufs=1, space="PSUM"))
            v_ps = pb.enter_context(tc.tile_pool(name="vps", bufs=1, space="PSUM"))

            def emit_vchunk(c):
                wv = wv_p.tile([128, KT, 256], F32R, tag="wv", name=f"wv{c}")
                nc.gpsimd.dma_start(
                    out=wv,
                    in_=wqkv_d[:, 2 * INNER + c * 256:2 * INNER + (c + 1) * 256]
                    .rearrange("(t p) n -> p t n", p=128))
                for ti in range(NT):
                    ps = v_ps.tile([128, 256], F32, tag="v", name=f"v{c}_{ti}")
                    for kt in range(KT):
                        nc.tensor.matmul(
                            ps, xnT[:, kt, ti * 128:(ti + 1) * 128],
                            wv[:, kt, :], start=(kt == 0), stop=(kt == KT - 1))
                    nc.vector.tensor_copy(
                        V_sb[:, ti, 4 * c:4 * (c + 1), 0:DHEAD],
                        ps.rearrange("p (a b) -> p a b", a=4))

            def emit_qkt(g):
                qkt = {}
                for qk_kind, m in (("q", g), ("k", 6 + g)):
                    wb = wqk_p.tile([128, KT, 128], F32R, tag="wqk")
                    nc.gpsimd.dma_start(
                        out=wb,
                        in_=wqkv_d[:, m * 128:(m + 1) * 128]
                        .rearrange("(t p) n -> p t n", p=128))
                    if qk_kind == "q":
                        nc.vector.tensor_scalar_mul(wb, wb, ALPHA * SCALE)
                    dst = qk_p.tile([128, N], F32R, tag="qkT", name=f"qkT{m}")
                    qkt[qk_kind] = dst
                    for ih in range(2):
                        ps = qk_ps.tile([128, 512], F32, tag="qk")
                        for kt in range(KT):
                            nc.tensor.matmul(
                                ps, wb[:, kt, :],
                                xnT[:, kt, ih * 512:(ih + 1) * 512],
                                start=(kt == 0), stop=(kt == KT - 1))
                        if ih == 0:
                            nc.vector.tensor_copy(dst[:, 0:512], ps)
                        else:
                            nc.scalar.copy(dst[:, 512:1024], ps)
                return qkt

            def stage1(hd, qkt):
                """h load -> raw -> blend (in place) -> blended store."""
                po = 64 * (hd % 2)
                bls = []
                for ti in range(NT):
                    bl = bl_p.tile([128, N], F32, tag="bl")
                    nc.sync.dma_start(
                        out=bl, in_=h_d[hd, ti * 128:(ti + 1) * 128, :])
                    for jh in range(2):
                        rp = raw_ps.tile([128, 512], F32, tag="raw")
                        nc.tensor.matmul(
                            rp,
                            qkt["q"][po:po + 64, ti * 128:(ti + 1) * 128],
                            qkt["k"][po:po + 64, jh * 512:(jh + 1) * 512],
                            start=True, stop=True)
                        nc.vector.scalar_tensor_tensor(
                            out=bl[:, jh * 512:(jh + 1) * 512],
                            in0=bl[:, jh * 512:(jh + 1) * 512], scalar=0.75,
                            in1=rp, op0=ALU.mult, op1=ALU.add)
                    nc.sync.dma_start(
                        out=blend_d[hd, ti * 128:(ti + 1) * 128, :], in_=bl)
                    bls.append(bl)
                return bls

            def stage2(hd, bls):
                """transpose -> exp -> PV -> deferred normalize into OT."""
                g, sub = divmod(hd, 2)
                po = 64 * sub
                pvt = pv_ps.tile([DHEAD + 1, 2, 512], F32, tag="pv")
                for jt in range(NT):
                    for ih in range(2):
                        tp = trb_ps.tile([128, 512], F32, tag="trb")
                        for tc_ in range(4):
                            ti = 4 * ih + tc_
                            nc.tensor.transpose(
                                tp[:, tc_ * 128:(tc_ + 1) * 128],
                                bls[ti][:, jt * 128:(jt + 1) * 128], ident)
                        at = at_p.tile([128, 512], F32R, tag="at")
                        nc.scalar.activation(out=at, in_=tp, func=AF.Exp)
                        nc.tensor.matmul(
                            pvt[:, ih, :], V_sb[:, jt, hd, :], at,
                            start=(jt == 0), stop=(jt == NT - 1))

                # evacuate PV psum immediately (pv bufs=1): unnormalized OT +
                # reciprocal row; the deferred in-place multiply by the
                # broadcast 1/denom lands whenever the DMA bounce returns and
                # does not block the next head's PV.
                nc.vector.tensor_copy(OT[po:po + 64, g, :], pvt[0:DHEAD, :, :])
                rd = sm_p.tile([1, N], F32, tag="rd")
                nc.vector.reciprocal(out=rd, in_=pvt[DHEAD:DHEAD + 1, :, :])
                nc.sync.dma_start(
                    out=rdsc_d[hd, :].rearrange("(a b) -> a b", a=1), in_=rd)
                rdb = sm_p.tile([128, N], F32, tag="rdb")
                _sc = rdsc_d[hd, :]
                nc.gpsimd.dma_start(out=rdb, in_=bass.AP(
                    tensor=_sc.tensor, offset=_sc.offset,
                    ap=[[0, 128]] + list(_sc.ap)))
                nc.vector.tensor_mul(OT[po:po + 64, g, :],
                                     OT[po:po + 64, g, :],
                                     rdb[po:po + 64, :])

            # software pipeline: stage1 runs one head ahead of stage2 so the
            # DVE blend burst of head h+1 overlaps the PE/ACT transpose+exp
            # burst of head h.
            emit_vchunk(0)
            qkts = {0: emit_qkt(0)}
            bls_next = stage1(0, qkts[0]) if KHEADS else None
            for hd in range(KHEADS):
                bls_cur = bls_next
                nh = hd + 1
                if nh < KHEADS:
                    if nh % 2 == 1 and (nh + 1) // 2 < KHEADS // 2:
                        qkts[(nh + 1) // 2] = emit_qkt((nh + 1) // 2)
                    if nh % 4 == 0 and nh // 4 < 3:
                        emit_vchunk(nh // 4)
                    bls_next = stage1(nh, qkts[nh // 2])
                stage2(hd, bls_cur)

        # ------- output projection (wout streamed in 256-col chunks) -------
        with ExitStack() as pc:
            o_p = pc.enter_context(tc.tile_pool(name="osb", bufs=4))
            o_ps = pc.enter_context(tc.tile_pool(name="ops", bufs=3, space="PSUM"))

            for c in range(3):
                woc = wv_p.tile([128, KT, 256], F32R, tag="wv", name=f"wo{c}")
                nc.gpsimd.dma_start(
                    out=woc,
                    in_=wout_d[:, c * 256:(c + 1) * 256]
                    .rearrange("(t p) n -> p t n", p=128))
                for ti in range(NT):
                    op = o_ps.tile([128, 256], F32, tag="o")
                    for ft in range(KT):
                        nc.tensor.matmul(
                            op, OT[:, ft, ti * 128:(ti + 1) * 128],
                            woc[:, ft, :], start=(ft == 0), stop=(ft == KT - 1))
                    ot = o_p.tile([128, 256], F32, tag="ot")
                    nc.vector.scalar_tensor_tensor(
                        out=ot, in0=op, scalar=1.0,
                        in1=bob[:, c * 256:(c + 1) * 256],
                        op0=ALU.mult, op1=ALU.add)
                    nc.sync.dma_start(
                        out=out_d[ti * 128:(ti + 1) * 128, c * 256:(c + 1) * 256],
                        in_=ot)


def kernel(x, h, gamma, beta, w_qkv, w_out, b_out):
    global _cached_nc, LAST_RESULTS
    x = np.asarray(x, dtype=np.float32)
    h = np.asarray(h, dtype=np.float32)
    gamma = np.asarray(gamma, dtype=np.float32)
    beta = np.asarray(beta, dtype=np.float32)
    w_qkv = np.asarray(w_qkv, dtype=np.float32)
    w_out = np.asarray(w_out, dtype=np.float32)
    b_out = np.asarray(b_out, dtype=np.float32)

    if _cached_nc is None:
        _cached_nc = _build_nc()
    nc = _cached_nc

    in_maps = []
    for b in range(B):
        in_maps.append({
            "x": np.ascontiguousarray(x[b]),
            "h": np.ascontiguousarray(h[b]),
            "gamma": gamma, "beta": beta,
            "w_qkv": w_qkv, "w_out": w_out, "b_out": b_out,
        })
    res = run_bass_kernel_spmd(nc, in_maps, list(range(B)), trace=TRACE)
    LAST_RESULTS = res
    out = np.stack([res.results[b]["out"] for b in range(B)])
    blended = np.stack([res.results[b]["blended"] for b in range(B)])
    return out, blended


# revision 26
# speedup vs baseline: 144.8000x; 1.0108x over previous
"""Trainium2 Bass kernel for nn_Attention_45406394253499.

Fused: LayerNorm -> QKV proj -> blended attention (0.25*qk*scale + 0.75*h)
-> softmax -> PV -> out proj.  Outputs (out, blended).

Sharding: data-parallel over batch. B=8 batches -> 8 NeuronCores, one batch
per core, no collectives.  Each core runs the identical program on its slice.

Core-local dataflow (fp32; matmul operands in float32r ~ tf32):
  x [1024,768] --LN--> xn --PE transpose--> xnT [768,1024] (feature-major)
  V = xnT.T @ w_qkv_v   [1024,768] token-major + ones col (denominator trick)
  per head-pair g (heads 2g, 2g+1):    <- interleaved so DMA/PE/DVE/ACT overlap
    qT_g,kT_g = w_qkv_block.T @ xnT    [128,1024] each (q pre-scaled 0.25/sqrt d)
    per head:
      raw      = qT.T @ kT             [128i,512j] PSUM tiles
      blended  = 0.75*h + raw          (DVE stt, evacuates PSUM) -> DRAM + SBUF
      per j-tile: PE-transpose 8x128x128 -> PSUM, ACT exp -> attnT [j,i] f32r,
                  2 PV matmuls accumulate [V|1].T @ attnT -> [65, 2, 512] PSUM
      OT_head = PV[0:64] * (1/PV[64])  (reciprocal broadcast via DRAM bounce)
  out = OT.T @ w_out + b_out           [1024,768]
"""

import numpy as np
from contextlib import ExitStack

import concourse.bass as bass
import concourse.tile as tile
from concourse import bacc, mybir
from concourse.bass_utils import run_bass_kernel_spmd
from concourse.masks import make_identity

B, N, DIM = 8, 1024, 768
HEADS, DHEAD = 12, 64
INNER = HEADS * DHEAD            # 768
ALPHA = 0.25
SCALE = DHEAD ** -0.5
EPS = 1e-5

F32 = mybir.dt.float32
F32R = mybir.dt.float32r
AF = mybir.ActivationFunctionType
ALU = mybir.AluOpType

NT = N // 128                    # 8 token tiles
KT = DIM // 128                  # 6 feature tiles

import os as _os
KHEADS = int(_os.environ.get("KHEADS", str(HEADS)))

TRACE = False
LAST_RESULTS = None
_cached_nc = None


def _build_nc() -> bass.Bass:
    nc = bacc.Bacc("TRN2", target_bir_lowering=False, debug=False)

    x_d = nc.declare_dram_parameter("x", [N, DIM], F32, isOutput=False)
    h_d = nc.declare_dram_parameter("h", [HEADS, N, N], F32, isOutput=False)
    gamma_d = nc.declare_dram_parameter("gamma", [DIM], F32, isOutput=False)
    beta_d = nc.declare_dram_parameter("beta", [DIM], F32, isOutput=False)
    wqkv_d = nc.declare_dram_parameter("w_qkv", [DIM, 3 * INNER], F32, isOutput=False)
    wout_d = nc.declare_dram_parameter("w_out", [INNER, DIM], F32, isOutput=False)
    bout_d = nc.declare_dram_parameter("b_out", [DIM], F32, isOutput=False)
    out_d = nc.declare_dram_parameter("out", [N, DIM], F32, isOutput=True)
    blend_d = nc.declare_dram_parameter("blended", [HEADS, N, N], F32, isOutput=True)
    # DRAM bounce buffer for broadcasting per-token 1/denom across partitions
    # (SBUF DMA sources cannot replicate; DRAM sources can). Declared as an
    # output: internal DRAM tensors fail NEFF load under the axon PJRT path.
    rdsc_d = nc.declare_dram_parameter("rd_scratch", [HEADS, N], F32,
                                       isOutput=True)

    with tile.TileContext(nc) as tc:
        _emit(tc, nc, x_d, h_d, gamma_d, beta_d, wqkv_d, wout_d, bout_d,
              out_d, blend_d, rdsc_d)
    nc.compile()
    return nc


def _emit(tc, nc, x_d, h_d, gamma_d, beta_d, wqkv_d, wout_d, bout_d,
          out_d, blend_d, rdsc_d):
    with ExitStack() as root:
        const = root.enter_context(tc.tile_pool(name="const", bufs=1))
        ident = const.tile([128, 128], F32)
        make_identity(nc, ident)
        eps_t = const.tile([128, 1], F32)
        nc.vector.memset(eps_t, EPS)
        gbT = const.tile([128, KT], F32)
        bbT = const.tile([128, KT], F32)
        bob = const.tile([128, DIM], F32)
        _bo = bout_d[:]
        nc.gpsimd.dma_start(out=bob, in_=bass.AP(
            tensor=_bo.tensor, offset=_bo.offset,
            ap=[[0, 128]] + list(_bo.ap)))
        nc.sync.dma_start(out=gbT, in_=gamma_d[:].rearrange("(t p) -> p t", p=128))
        nc.sync.dma_start(out=bbT, in_=beta_d[:].rearrange("(t p) -> p t", p=128))

        # long-lived tensors
        big = root.enter_context(tc.tile_pool(name="big", bufs=1))
        xnT = big.tile([128, KT, N], F32R)
        V_sb = big.tile([128, NT, HEADS, DHEAD + 1], F32R)
        OT = big.tile([128, KT, N], F32R)
        ones_t = const.tile([128, NT * HEADS], F32)
        nc.vector.memset(ones_t, 1.0)
        nc.vector.tensor_copy(
            V_sb[:, :, :, DHEAD], ones_t.rearrange("p (a b) -> p a b", a=NT))

        # attention-phase working pools (root scope so DMAs hoist early)
        qk_p = root.enter_context(tc.tile_pool(name="qk", bufs=4))
        bl_p = root.enter_context(tc.tile_pool(name="bl", bufs=17))
        at_p = root.enter_context(tc.tile_pool(name="at", bufs=4))
        sm_p = root.enter_context(tc.tile_pool(name="sm", bufs=2))
        wv_p = root.enter_context(tc.tile_pool(name="wv", bufs=2))

        # ---------------- LayerNorm -> xnT ----------------
        with ExitStack() as pl:
            x_p = pl.enter_context(tc.tile_pool(name="xin", bufs=2))
            ln_p = pl.enter_context(tc.tile_pool(name="ln", bufs=8))
            tr_ps = pl.enter_context(tc.tile_pool(name="lntr", bufs=2, space="PSUM"))
            for ti in range(NT):
                x_t = x_p.tile([128, DIM], F32, tag="x")
                nc.sync.dma_start(out=x_t, in_=x_d[ti * 128:(ti + 1) * 128, :])
                stats = ln_p.tile([128, 3, 6], F32, tag="st")
                for sg in range(3):
                    nc.vector.bn_stats(
                        out=stats[:, sg, :], in_=x_t[:, sg * 256:(sg + 1) * 256])
                mv = ln_p.tile([128, 2], F32, tag="mv")
                nc.vector.bn_aggr(out=mv, in_=stats)
                s_t = ln_p.tile([128, 1], F32, tag="s")
                nc.scalar.activation(out=s_t, in_=mv[:, 1:2], func=AF.Sqrt,
                                     bias=eps_t, scale=1.0)
                nc.vector.reciprocal(out=s_t, in_=s_t)
                tneg = ln_p.tile([128, 1], F32, tag="t")
                nc.vector.scalar_tensor_tensor(
                    out=tneg, in0=mv[:, 0:1], scalar=-1.0, in1=s_t,
                    op0=ALU.mult, op1=ALU.mult)
                xc = x_p.tile([128, DIM], F32, tag="xc")
                nc.scalar.activation(out=xc, in_=x_t, func=AF.Identity,
                                     bias=tneg, scale=s_t)
                for ft in range(KT):
                    tp = tr_ps.tile([128, 128], F32, tag="tp")
                    nc.tensor.transpose(tp, xc[:, ft * 128:(ft + 1) * 128], ident)
                    nc.vector.tensor_scalar(
                        out=xnT[:, ft, ti * 128:(ti + 1) * 128], in0=tp,
                        scalar1=gbT[:, ft:ft + 1], scalar2=bbT[:, ft:ft + 1],
                        op0=ALU.mult, op1=ALU.add)

        # ------- interleaved V-chunks + QKV + attention, per head pair -----
        with ExitStack() as pb:
            wqk_p = pb.enter_context(tc.tile_pool(name="wqk", bufs=2))
            qk_ps = pb.enter_context(tc.tile_pool(name="qkps", bufs=1, space="PSUM"))
            raw_ps = pb.enter_context(tc.tile_pool(name="raw", bufs=2, space="PSUM"))
            trb_ps = pb.enter_context(tc.tile_pool(name="trb", bufs=2, space="PSUM"))
            pv_ps = pb.enter_context(tc.tile_pool(name="pv", bufs=1, space="PSUM"))
            v_ps = pb.enter_context(tc.tile_pool(name="vps", bufs=1, space="PSUM"))

            def emit_vchunk(c):
                wv = wv_p.tile([128, KT, 256], F32R, tag="wv", name=f"wv{c}")
                nc.gpsimd.dma_start(
                    out=wv,
                    in_=wqkv_d[:, 2 * INNER + c * 256:2 * INNER + (c + 1) * 256]
                    .rearrange("(t p) n -> p t n", p=128))
                for ti in range(NT):
                    ps = v_ps.tile([128, 256], F32, tag="v", name=f"v{c}_{ti}")
                    for kt in range(KT):
                        nc.tensor.matmul(
                            ps, xnT[:, kt, ti * 128:(ti + 1) * 128],
                            wv[:, kt, :], start=(kt == 0), stop=(kt == KT - 1))
                    nc.vector.tensor_copy(
                        V_sb[:, ti, 4 * c:4 * (c + 1), 0:DHEAD],
                        ps.rearrange("p (a b) -> p a b", a=4))

            def emit_qkt(g):
                qkt = {}
                for qk_kind, m in (("q", g), ("k", 6 + g)):
                    wb = wqk_p.tile([128, KT, 128], F32R, tag="wqk")
                    nc.gpsimd.dma_start(
                        out=wb,
                        in_=wqkv_d[:, m * 128:(m + 1) * 128]
                        .rearrange("(t p) n -> p t n", p=128))
                    if qk_kind == "q":
                        nc.vector.tensor_scalar_mul(wb, wb, ALPHA * SCALE)
                    dst = qk_p.tile([128, N], F32R, tag="qkT", name=f"qkT{m}")
                    qkt[qk_kind] = dst
                    for ih in range(2):
                        ps = qk_ps.tile([128, 512], F32, tag="qk")
                        for kt in range(KT):
                            nc.tensor.matmul(
                                ps, wb[:, kt, :],
                                xnT[:, kt, ih * 512:(ih + 1) * 512],
                                start=(kt == 0), stop=(kt == KT - 1))
                        if ih == 0:
                            nc.vector.tensor_copy(dst[:, 0:512], ps)
                        else:
                            nc.scalar.copy(dst[:, 512:1024], ps)
                return qkt

            def stage1(hd, qkt):
                """h load -> raw -> blend (in place) -> blended store."""
                po = 64 * (hd % 2)
                bls = []
                for ti in range(NT):
                    bl = bl_p.tile([128, N], F32, tag="bl")
                    nc.sync.dma_start(
                        out=bl, in_=h_d[hd, ti * 128:(ti + 1) * 128, :])
                    for jh in range(2):
                        rp = raw_ps.tile([128, 512], F32, tag="raw")
                        nc.tensor.matmul(
                            rp,
                            qkt["q"][po:po + 64, ti * 128:(ti + 1) * 128],
                            qkt["k"][po:po + 64, jh * 512:(jh + 1) * 512],
                            start=True, stop=True)
                        nc.vector.scalar_tensor_tensor(
                            out=bl[:, jh * 512:(jh + 1) * 512],
                            in0=bl[:, jh * 512:(jh + 1) * 512], scalar=0.75,
                            in1=rp, op0=ALU.mult, op1=ALU.add)
                    nc.sync.dma_start(
                        out=blend_d[hd, ti * 128:(ti + 1) * 128, :], in_=bl)
                    bls.append(bl)
                return bls

            def stage2(hd, bls):
                """transpose -> exp -> PV -> deferred normalize into OT."""
                g, sub = divmod(hd, 2)
                po = 64 * sub
                pvt = pv_ps.tile([DHEAD + 1, 2, 512], F32, tag="pv")
                for jt in range(NT):
                    for ih in range(2):
                        tp = trb_ps.tile([128, 512], F32, tag="trb")
                        for tc_ in range(4):
                            ti = 4 * ih + tc_
                            nc.tensor.transpose(
                                tp[:, tc_ * 128:(tc_ + 1) * 128],
                                bls[ti][:, jt * 128:(jt + 1) * 128], ident)
                        at = at_p.tile([128, 512], F32R, tag="at")
                        nc.scalar.activation(out=at, in_=tp, func=AF.Exp)
                        nc.tensor.matmul(
                            pvt[:, ih, :], V_sb[:, jt, hd, :], at,
                            start=(jt == 0), stop=(jt == NT - 1))

                # evacuate PV psum immediately (pv bufs=1): unnormalized OT +
                # reciprocal row; the deferred in-place multiply by the
                # broadcast 1/denom lands whenever the DMA bounce returns and
                # does not block the next head's PV.
                nc.vector.tensor_copy(OT[po:po + 64, g, :], pvt[0:DHEAD, :, :])
                rd = sm_p.tile([1, N], F32, tag="rd", bufs=1)
                nc.vector.reciprocal(out=rd, in_=pvt[DHEAD:DHEAD + 1, :, :])
                nc.sync.dma_start(
                    out=rdsc_d[hd, :].rearrange("(a b) -> a b", a=1), in_=rd)
                rdb = sm_p.tile([128, N], F32, tag="rdb")
                _sc = rdsc_d[hd, :]
                nc.gpsimd.dma_start(out=rdb[po:po + 64, :], in_=bass.AP(
                    tensor=_sc.tensor, offset=_sc.offset,
                    ap=[[0, 64]] + list(_sc.ap)))
                nc.vector.tensor_mul(OT[po:po + 64, g, :],
                                     OT[po:po + 64, g, :],
                                     rdb[po:po + 64, :])

            # software pipeline: stage1 runs one head ahead of stage2 so the
            # DVE blend burst of head h+1 overlaps the PE/ACT transpose+exp
            # burst of head h.
            emit_vchunk(0)
            qkts = {0: emit_qkt(0)}
            bls_next = stage1(0, qkts[0]) if KHEADS else None
            for hd in range(KHEADS):
                bls_cur = bls_next
                nh = hd + 1
                if nh < KHEADS:
                    if nh % 2 == 1 and (nh + 1) // 2 < KHEADS // 2:
                        qkts[(nh + 1) // 2] = emit_qkt((nh + 1) // 2)
                    if nh % 4 == 0 and nh // 4 < 3:
                        emit_vchunk(nh // 4)
                    bls_next = stage1(nh, qkts[nh // 2])
                stage2(hd, bls_cur)

        # ------- output projection (wout streamed in 256-col chunks) -------
        with ExitStack() as pc:
            o_p = pc.enter_context(tc.tile_pool(name="osb", bufs=4))
            o_ps = pc.enter_context(tc.tile_pool(name="ops", bufs=3, space="PSUM"))

            for c in range(3):
                woc = wv_p.tile([128, KT, 256], F32R, tag="wv", name=f"wo{c}")
                nc.gpsimd.dma_start(
                    out=woc,
                    in_=wout_d[:, c * 256:(c + 1) * 256]
                    .rearrange("(t p) n -> p t n", p=128))
                for ti in range(NT):
                    op = o_ps.tile([128, 256], F32, tag="o")
                    for ft in range(KT):
                        nc.tensor.matmul(
                            op, OT[:, ft, ti * 128:(ti + 1) * 128],
                            woc[:, ft, :], start=(ft == 0), stop=(ft == KT - 1))
                    ot = o_p.tile([128, 256], F32, tag="ot")
                    nc.vector.scalar_tensor_tensor(
                        out=ot, in0=op, scalar=1.0,
                        in1=bob[:, c * 256:(c + 1) * 256],
                        op0=ALU.mult, op1=ALU.add)
                    nc.sync.dma_start(
                        out=out_d[ti * 128:(ti + 1) * 128, c * 256:(c + 1) * 256],
                        in_=ot)


def kernel(x, h, gamma, beta, w_qkv, w_out, b_out):
    global _cached_nc, LAST_RESULTS
    x = np.asarray(x, dtype=np.float32)
    h = np.asarray(h, dtype=np.float32)
    gamma = np.asarray(gamma, dtype=np.float32)
    beta = np.asarray(beta, dtype=np.float32)
    w_qkv = np.asarray(w_qkv, dtype=np.float32)
    w_out = np.asarray(w_out, dtype=np.float32)
    b_out = np.asarray(b_out, dtype=np.float32)

    if _cached_nc is None:
        _cached_nc = _build_nc()
    nc = _cached_nc

    in_maps = []
    for b in range(B):
        in_maps.append({
            "x": np.ascontiguousarray(x[b]),
            "h": np.ascontiguousarray(h[b]),
            "gamma": gamma, "beta": beta,
            "w_qkv": w_qkv, "w_out": w_out, "b_out": b_out,
        })
    res = run_bass_kernel_spmd(nc, in_maps, list(range(B)), trace=TRACE)
    LAST_RESULTS = res
    out = np.stack([res.results[b]["out"] for b in range(B)])
    blended = np.stack([res.results[b]["blended"] for b in range(B)])
    return out, blended
